# revision 75
# baseline (speedup 1.0000x reference)
"""Dense GAT layer (attention + out-proj + residual + LayerNorm + SiLU + node mask)
as a fused Bass/Tile kernel on 8 Trainium2 NeuronCores.

Sharding: core = (b, half) with b = core//2, half = core%2. Each core computes
output rows [half*1024, (half+1)*1024) of batch b: it builds K/V for the full
2048 keys of its batch and Q only for its 1024 query rows, so no cross-core
reduction is needed — the host just concatenates row blocks.

Key structure (per core):
  * Loop order is query-chunk OUTER, head inner: all projections happen while
    streaming the first query chunk, and the output projection + LayerNorm for
    the first 512 rows overlaps the second chunk's attention (no serial tail).
  * S + adjacency mask fused in ONE fp8 DoubleRow matmul: slab0 = K^T.T@Q^T
    (d-contraction), slab1 = I.T@maskbias (adds 0 / -240 per pair). Both
    slabs are step-sliced views of two big SBUF tiles holding
    [qt_h..., maskbias_mc...] and [kt_(h,mc)..., ident]. exp of a -240-biased
    score underflows to zero, so no separate mask multiply exists anywhere.
  * softmax exp is split between ACT (hardware Exp -> fp8, scaled 1/8 via
    bias=-ln8) and DVE (Schraudolph: i16 = s*SCALE*128*log2e + (16256-384),
    bitcast bf16 == exp(s*SCALE)/8 with ~2% sawtooth that cancels in the row
    normalization).
  * A@V is DoubleRow fp8 (ACT groups) / classic bf16 (DVE groups), with a
    ones-column appended to V so O and the softmax row-sums come from the
    same accumulation.
  * SiLU uses Exp (1/(1+e^-x)) instead of Sigmoid so the whole kernel needs a
    single ACT function table (natural_log_exp) — no mid-kernel table loads.
node_mask only gates query rows (self-loops guarantee non-empty rows), so it
reduces to the final elementwise multiply.
"""

import math
from functools import lru_cache

import ml_dtypes
import numpy as np

import concourse.bacc as bacc
import concourse.mybir as mybir
import concourse.tile as tile
from concourse import masks

B, N, F = 4, 2048, 128
H, D = 8, 128
NQ = 1024
NCORES = 8
EPS = 1e-5
SCALE = 1.0 / math.sqrt(D)
LOG2E = 1.4426950408889634

F32 = mybir.dt.float32
F32R = mybir.dt.float32r
BF16 = mybir.dt.bfloat16
F8 = mybir.dt.float8e4
I16 = mybir.dt.int16
AF = mybir.ActivationFunctionType
ALU = mybir.AluOpType
DR = mybir.MatmulPerfMode.DoubleRow

NMC = N // 128   # 16 m-chunks
NCS = NQ // 512  # 2 query-column chunks
NG = 2           # m-chunks per group
NGRP = NMC // NG

DVE_GROUPS = (2, 5, 7)
ACT_CHUNKS = [mc for mc in range(NMC) if (mc // NG) not in DVE_GROUPS]
DVE_CHUNKS = [mc for mc in range(NMC) if (mc // NG) in DVE_GROUPS]
VA_SLOT = {mc: i for i, mc in enumerate(ACT_CHUNKS)}
VB_SLOT = {mc: i for i, mc in enumerate(DVE_CHUNKS)}

SCH_MUL = SCALE * 128.0 * LOG2E
SCH_ADD = 16256.0 - 384.0
EXP_BIAS = -math.log(8.0)

NSLOT_Q = H            # QM slots 0..7 = qt per head, 8..23 = maskbias chunks
NSLOT_QM = H + NMC
NSLOT_K = H * NMC      # K8 slots h*16+mc = kt, slot 128 = identity


def _build_program(affine: bool = False):
    nc = bacc.Bacc(
        "TRN2", target_bir_lowering=False, debug=False, num_devices=NCORES
    )
    d_xT = nc.declare_dram_parameter("xT", [F, N], F32R, isOutput=False)
    d_xqT = nc.declare_dram_parameter("xqT", [F, NQ], F32R, isOutput=False)
    d_xres = nc.declare_dram_parameter("xres", [128, 8, 128], F32, isOutput=False)
    d_maskb = nc.declare_dram_parameter("maskb", [128, NMC, NQ], F8, isOutput=False)
    d_wq = nc.declare_dram_parameter("wq", [F, H * D], F32R, isOutput=False)
    d_wk = nc.declare_dram_parameter("wk", [F, H * D], F32R, isOutput=False)
    d_wv = nc.declare_dram_parameter("wv", [F, H * D], F32R, isOutput=False)
    d_wo = nc.declare_dram_parameter("wo", [128, 8, 128], BF16, isOutput=False)
    d_gb = nc.declare_dram_parameter("gb", [2, 128], F32, isOutput=False)
    d_nm = nc.declare_dram_parameter("nm", [128, 8], F32, isOutput=False)
    d_out = nc.declare_dram_parameter("out", [128, 8, 128], F32, isOutput=True)

    with tile.TileContext(nc) as tc:
        with (
            tc.tile_pool(name="const", bufs=1) as const,
            tc.tile_pool(name="small", bufs=4) as sp,
        ):
            # DMA order: first S group needs wk, xT slice 0, wq, xqT, mask 0-1.
            wk = const.tile([128, H * D], F32R)
            nc.sync.dma_start(wk[:], d_wk[:])
            xT = const.tile([128, N], F32R)
            nc.sync.dma_start(xT[:, 0:512], d_xT[:, 0:512])
            wq = const.tile([128, H * D], F32R)
            nc.sync.dma_start(wq[:], d_wq[:])
            xqT = const.tile([128, NQ], F32R)
            nc.sync.dma_start(xqT[:], d_xqT[:])
            qm = const.tile([128, NSLOT_QM * NQ], F8, tag="qm")
            qmv = qm[:].rearrange("p (c n) -> p c n", n=NQ)

            def dma_mask(a, b):
                nc.sync.dma_start(qmv[:, H + a:H + b, :], d_maskb[:, a:b, :])

            dma_mask(0, 2)
            dma_mask(2, 4)
            wv = const.tile([128, H * D], F32R)
            nc.sync.dma_start(wv[:], d_wv[:])
            nc.sync.dma_start(xT[:, 512:1024], d_xT[:, 512:1024])
            dma_mask(4, 8)
            dma_mask(8, 12)
            nc.sync.dma_start(xT[:, 1024:1536], d_xT[:, 1024:1536])
            nc.sync.dma_start(xT[:, 1536:2048], d_xT[:, 1536:2048])
            dma_mask(12, 16)
            wo = const.tile([128, 8 * 128], BF16)
            wo_v = wo[:].rearrange("p (h d) -> p h d", d=128)
            nc.sync.dma_start(wo_v, d_wo[:])

            if affine:
                gbg = const.tile([1, 128], F32)
                nc.sync.dma_start(gbg[:], d_gb[0:1, :])
                gbb = const.tile([1, 128], F32)
                nc.sync.dma_start(gbb[:], d_gb[1:2, :])
            nm = const.tile([128, 8], F32)
            nc.sync.dma_start(nm[:], d_nm[:])
            xres = const.tile([128, 8 * 128], F32)
            xres_v = xres[:].rearrange("p (c d) -> p c d", d=128)
            nc.sync.dma_start(xres_v, d_xres[:])

            ident = const.tile([128, 128], BF16)
            masks.make_identity(nc, ident[:])
            k8 = const.tile([128, (NSLOT_K + 1) * 128], F8, tag="k8")
            k8v = k8[:].rearrange("p (c n) -> p c n", n=128)
            nc.vector.tensor_copy(k8v[:, NSLOT_K, :], ident[:])
            ones1 = const.tile([1, 128], F32)
            nc.vector.memset(ones1[:], 1.0)
            eps_t = const.tile([128, 1], F32)
            nc.vector.memset(eps_t[:], EPS)
            expb = const.tile([128, 1], F32)
            nc.vector.memset(expb[:], EXP_BIAS)
            one_col = const.tile([128, 1], F32)
            nc.vector.memset(one_col[:], 1.0)

            va8 = const.tile([128, len(ACT_CHUNKS) * 8 * 130], F8)
            va8_v = va8[:].rearrange("p (c h k) -> p c h k", h=8, k=130)
            vb16 = const.tile([128, len(DVE_CHUNKS) * 8 * 130], BF16)
            vb16_v = vb16[:].rearrange("p (c h k) -> p c h k", h=8, k=130)
            nc.vector.memset(va8_v[:, :, :, 128:129], 1.0)
            nc.vector.memset(vb16_v[:, :, :, 128:129], 1.0)

            OT = const.tile([128, H * NQ], BF16)  # O^T (unnormalized)
            OT_v = OT[:].rearrange("p (h n) -> p h n", n=NQ)

            if affine:
                gamma_bc = const.tile([128, 128], F32)
                beta_bc = const.tile([128, 128], F32)

            with (
                tc.tile_pool(name="hp", bufs=4) as hp,
                tc.tile_pool(name="pp", bufs=4) as pp,
                tc.tile_pool(name="ps_m", bufs=1, space="PSUM") as ps_m,
                tc.tile_pool(name="ps_s", bufs=2, space="PSUM") as ps_s,
                tc.tile_pool(name="ps_av", bufs=1, space="PSUM") as ps_av,
                tc.tile_pool(name="ps_o", bufs=1, space="PSUM") as ps_o,
            ):
                if affine:
                    gps = ps_m.tile([128, 512], F32, tag="pkq")
                    nc.tensor.matmul(gps[:, 0:128], ones1[:], gbg[:],
                                     start=True, stop=True)
                    nc.tensor.matmul(gps[:, 128:256], ones1[:], gbb[:],
                                     start=True, stop=True)
                    nc.vector.tensor_copy(gamma_bc[:], gps[:, 0:128])
                    nc.vector.tensor_copy(beta_bc[:], gps[:, 128:256])

                def emit_kproj(h, j):
                    pk = ps_m.tile([128, 512], F32, tag="pkq")
                    nc.tensor.matmul(pk[:], wk[:, h * 128:(h + 1) * 128],
                                     xT[:, j * 512:(j + 1) * 512],
                                     start=True, stop=True)
                    nc.scalar.copy(
                        k8[:, (h * NMC + j * 4) * 128:(h * NMC + j * 4 + 4) * 128],
                        pk[:])

                def emit_qproj(h, j):
                    pq = ps_m.tile([128, 512], F32, tag="pkq")
                    nc.tensor.matmul(pq[:], wq[:, h * 128:(h + 1) * 128],
                                     xqT[:, j * 512:(j + 1) * 512],
                                     start=True, stop=True)
                    nc.vector.tensor_copy(qmv[:, h, j * 512:(j + 1) * 512], pq[:])

                def emit_vproj(mc):
                    pv = ps_s.tile([128, 1024], F32, tag="sg")
                    # two matmuls: a single output may not cross a PSUM bank
                    for vj in range(2):
                        nc.tensor.matmul(pv[:, vj * 512:(vj + 1) * 512],
                                         xT[:, mc * 128:(mc + 1) * 128],
                                         wv[:, vj * 512:(vj + 1) * 512],
                                         start=True, stop=True)
                    pv_v = pv[:].rearrange("p (h d) -> p h d", d=128)
                    if mc in VA_SLOT:
                        dst = va8_v[:, VA_SLOT[mc], :, 0:128]
                    else:
                        dst = vb16_v[:, VB_SLOT[mc], :, 0:128]
                    if mc % 2 == 0:
                        nc.scalar.copy(dst, pv_v)
                    else:
                        nc.vector.tensor_copy(dst, pv_v)

                def emit_s(qc, h, g):
                    qsl = slice(qc * 512, (qc + 1) * 512)
                    sg = ps_s.tile([128, NG * 512], F32, tag="sg")
                    sg_v = sg[:].rearrange("p (c n) -> p c n", n=512)
                    for c in range(NG):
                        mc = g * NG + c
                        ks = h * NMC + mc
                        nc.tensor.matmul(
                            sg_v[:, c, :],
                            k8v[:, ks:NSLOT_K + 1:(NSLOT_K - ks), :],
                            qmv[:, h:H + mc + 1:(H + mc - h), qsl],
                            start=True, stop=True, perf_mode=DR,
                        )
                    return sg, sg_v

                def ln_half(hf, po, po_v):
                    c3 = [128, 4, 128]
                    cs = slice(hf * 4, (hf + 1) * 4)
                    fo = sp.tile([128, 4 * 128], F32, tag="fo")
                    fo_v = fo[:].rearrange("p (c d) -> p c d", d=128)
                    nc.vector.tensor_tensor(fo_v, po_v, xres_v[:, cs, :],
                                            ALU.add)
                    # E[x] on DVE in parallel with x^2 on ACT
                    mu = sp.tile([128, 4], F32, tag="mu")
                    nc.vector.tensor_reduce(mu[:], fo_v, mybir.AxisListType.X,
                                            ALU.add)
                    sq = sp.tile([128, 4 * 128], F32, tag="sq")
                    nc.scalar.square(sq[:], fo[:])
                    sq_v = sq[:].rearrange("p (c d) -> p c d", d=128)
                    vs = sp.tile([128, 4], F32, tag="vs")
                    nc.vector.tensor_reduce(vs[:], sq_v, mybir.AxisListType.X,
                                            ALU.add)
                    mean = sp.tile([128, 4], F32, tag="mean")
                    nc.vector.tensor_scalar_mul(mean[:], mu[:], 1.0 / 128.0)
                    msq = sp.tile([128, 4], F32, tag="msq")
                    nc.vector.tensor_tensor(msq[:], mean[:], mean[:], ALU.mult)
                    # var = E[x^2] - mean^2 + eps
                    ex2 = sp.tile([128, 4], F32, tag="ex2")
                    nc.vector.tensor_scalar(ex2[:], vs[:], 1.0 / 128.0, EPS,
                                            ALU.mult, ALU.add)
                    var = sp.tile([128, 4], F32, tag="var")
                    nc.vector.tensor_tensor(var[:], ex2[:], msq[:],
                                            ALU.subtract)
                    # rsqrt(var) via bf16 bit-trick seed + one Newton step
                    vb = sp.tile([128, 4], BF16, tag="vb")
                    nc.vector.tensor_copy(vb[:], var[:])
                    yi = sp.tile([128, 4], I16, tag="yi")
                    nc.vector.tensor_scalar(yi[:], vb[:].bitcast(I16), -0.5,
                                            24375.0, ALU.mult, ALU.add)
                    y0 = sp.tile([128, 4], F32, tag="y0")
                    nc.vector.tensor_copy(y0[:], yi[:].bitcast(BF16))
                    yy = sp.tile([128, 4], F32, tag="yy")
                    nc.vector.tensor_tensor(yy[:], y0[:], y0[:], ALU.mult)
                    vyy = sp.tile([128, 4], F32, tag="vyy")
                    nc.vector.tensor_tensor(vyy[:], var[:], yy[:], ALU.mult)
                    nwt = sp.tile([128, 4], F32, tag="nwt")
                    nc.vector.tensor_scalar(nwt[:], vyy[:], -0.5, 1.5,
                                            ALU.mult, ALU.add)
                    rs = sp.tile([128, 4], F32, tag="rs")
                    nc.vector.tensor_tensor(rs[:], y0[:], nwt[:], ALU.mult)
                    # fused normalize per chunk: nrm = fo*rs - (mean*rs)
                    mrs = sp.tile([128, 4], F32, tag="mrs")
                    nc.vector.tensor_tensor(mrs[:], mean[:], rs[:], ALU.mult)
                    nmrs = sp.tile([128, 4], F32, tag="nmrs")
                    nc.vector.tensor_scalar_mul(nmrs[:], mrs[:], -1.0)
                    nrm = sp.tile([128, 4 * 128], F32, tag="nrm")
                    nrm_v = nrm[:].rearrange("p (c d) -> p c d", d=128)
                    for c4 in range(4):
                        nc.vector.tensor_scalar(
                            nrm_v[:, c4, :], fo_v[:, c4, :],
                            rs[:, c4:c4 + 1], nmrs[:, c4:c4 + 1],
                            ALU.mult, ALU.add)
                    if affine:
                        g1 = sp.tile([128, 4 * 128], F32, tag="g1")
                        g1_v = g1[:].rearrange("p (c d) -> p c d", d=128)
                        nc.vector.tensor_tensor(
                            g1_v, nrm_v,
                            gamma_bc[:].unsqueeze(1).broadcast_to(c3), ALU.mult)
                        g2 = sp.tile([128, 4 * 128], F32, tag="g2")
                        g2_v = g2[:].rearrange("p (c d) -> p c d", d=128)
                        nc.vector.tensor_tensor(
                            g2_v, g1_v,
                            beta_bc[:].unsqueeze(1).broadcast_to(c3), ALU.add)
                    else:
                        g2, g2_v = nrm, nrm_v
                    gn = sp.tile([128, 4 * 128], F32, tag="gn")
                    gn_v = gn[:].rearrange("p (c d) -> p c d", d=128)
                    nc.vector.tensor_tensor(
                        gn_v, g2_v, nm[:, cs].unsqueeze(-1).broadcast_to(c3),
                        ALU.mult)
                    fin = sp.tile([128, 4 * 128], F32, tag="fin")
                    fin_v = fin[:].rearrange("p (c d) -> p c d", d=128)
                    if hf == 0:
                        # mid-kernel: SiLU via Exp so the ACT exp table stays
                        # loaded for the surrounding attention stream
                        ex = sp.tile([128, 4 * 128], F32, tag="ex")
                        nc.scalar.activation(ex[:], g2[:], AF.Exp, scale=-1.0)
                        ep = sp.tile([128, 4 * 128], F32, tag="ep")
                        nc.scalar.activation(ep[:], ex[:], AF.Identity,
                                             bias=one_col[:])
                        rc = sp.tile([128, 4 * 128], F32, tag="rc")
                        nc.vector.reciprocal(rc[:], ep[:])
                        nc.vector.tensor_tensor(fin_v, gn_v, rc[:].rearrange(
                            "p (c d) -> p c d", d=128), ALU.mult)
                    else:
                        # kernel end: real Sigmoid (table switch overlaps the
                        # preceding DVE chain; nothing needs exp afterwards)
                        sg2 = sp.tile([128, 4 * 128], F32, tag="sg2")
                        nc.scalar.activation(sg2[:], g2[:], AF.Sigmoid)
                        nc.vector.tensor_tensor(fin_v, gn_v, sg2[:].rearrange(
                            "p (c d) -> p c d", d=128), ALU.mult)
                    nc.sync.dma_start(d_out[:, cs, :], fin_v)

                # head-0: only the chunks the first S group needs go up
                # front; the rest slot into task 0 behind the S lookahead
                emit_kproj(0, 0)
                emit_qproj(0, 0)
                emit_qproj(0, 1)

                po_cur = [None, None]  # (tile, view) for the active qc

                def flush_block(blk):
                    # transpose the previous (qc, h)'s O block and fold it
                    # into the output projection incrementally; deferred so
                    # these PE ops sit behind the next head's S groups
                    # instead of stalling the exp stream.
                    fqc, fh, foh_v = blk
                    tpf = ps_m.tile([128, 512], F32, tag="pkq")
                    tp = tpf[:, 0:256].bitcast(BF16)
                    for s4 in range(4):
                        nc.tensor.matmul(
                            tp[:, s4 * 128:(s4 + 1) * 128],
                            foh_v[:, s4, :], ident[:],
                            is_transpose=True, start=True, stop=True,
                        )
                    otc = OT_v[:, fh, fqc * 512:(fqc + 1) * 512]
                    nc.vector.tensor_copy(otc, tp[:])
                    if fh == 0:
                        po = ps_o.tile([128, 512], F32, tag="po")
                        po_cur[0] = po
                        po_cur[1] = po[:].rearrange("p (c d) -> p c d", d=128)
                    po = po_cur[0]
                    for c4 in range(4):
                        c = fqc * 4 + c4
                        # start/stop are bank-granular: only the first/last
                        # matmul touching the bank may carry them.
                        nc.tensor.matmul(
                            po[:, c4 * 128:(c4 + 1) * 128],
                            OT_v[:, fh, c * 128:(c + 1) * 128],
                            wo_v[:, fh, :],
                            start=(fh == 0 and c4 == 0),
                            stop=(fh == H - 1 and c4 == 3),
                        )

                tasks = [(qc, h, g) for qc in range(NCS) for h in range(H)
                         for g in range(NGRP)]
                pend = emit_s(*tasks[0])
                deferred = None
                for i, (qc, h, g) in enumerate(tasks):
                    sg, sg_v = pend
                    is_dve = g in DVE_GROUPS
                    if is_dve:
                        if qc == 1 and h == H - 1:
                            # last head: DVE is winding down; ACT (idle at the
                            # tail) does the exp, still bf16 for the bf16 AV
                            pt16 = pp.tile([128, NG * 512], BF16, tag="pt16")
                            nc.scalar.activation(
                                pt16[:].rearrange("p (c n) -> p c n", n=512),
                                sg_v, AF.Exp, bias=expb[:], scale=SCALE)
                            praw_v = pt16[:].rearrange(
                                "p (c n) -> p c n", n=512)
                        else:
                            pti = pp.tile([128, NG * 512], I16, tag="pti")
                            nc.vector.tensor_scalar(pti[:], sg[:], SCH_MUL,
                                                    SCH_ADD, ALU.mult, ALU.add)
                            praw_v = pti[:].bitcast(BF16).rearrange(
                                "p (c n) -> p c n", n=512)
                    else:
                        pt8 = pp.tile([128, NG * 512], F8, tag="pt8")
                        pt8_v = pt8[:].rearrange("p (c n) -> p c n", n=512)
                        nc.scalar.activation(pt8_v, sg_v, AF.Exp,
                                             bias=expb[:], scale=SCALE)
                    # PE lookahead: next S group, then interleaved proj work
                    if i + 1 < len(tasks):
                        pend = emit_s(*tasks[i + 1])
                    if g == 1 and deferred is not None:
                        flush_block(deferred)
                        deferred = None
                        if qc == 1 and h == 0:
                            # half 0 is fully projected now; finalize it while
                            # qc1 attention streams
                            ln_half(0, po_cur[0], po_cur[1])
                    if qc == 0:
                        if h == 0:
                            if g == 0:
                                for j in range(1, 4):
                                    emit_kproj(0, j)
                            emit_vproj(g * 2)
                            emit_vproj(g * 2 + 1)
                        if h + 1 < H:
                            if 1 <= g <= 4:
                                emit_kproj(h + 1, g - 1)
                            if g == 2:
                                emit_qproj(h + 1, 0)
                            elif g == 3:
                                emit_qproj(h + 1, 1)
                    if g == 0:
                        avA = ps_av.tile([128, 512], F32, tag="avA")
                        avB = ps_av.tile([128, 512], F32, tag="avB")
                        oh = hp.tile([128, 4 * 128], BF16, tag="oh")
                        oh_v = oh[:].rearrange("p (s d) -> p s d", d=128)
                    if is_dve:
                        # c-outer: the first chunk's four AVs need only the
                        # first Schraudolph half
                        for c in range(NG):
                            mc = g * NG + c
                            for s in range(4):
                                av = avA if s < 2 else avB
                                off = (s % 2) * 256
                                nc.tensor.matmul(
                                    av[:, off:off + 129],
                                    praw_v[:, c, s * 128:(s + 1) * 128],
                                    vb16_v[:, VB_SLOT[mc], h, 0:129],
                                    start=(g == 0 and s % 2 == 0 and c == 0),
                                    stop=(g == NGRP - 1 and s % 2 == 1
                                          and c == NG - 1),
                                )
                    else:
                        for s in range(4):
                            av = avA if s < 2 else avB
                            off = (s % 2) * 256
                            slot = VA_SLOT[g * NG]
                            nc.tensor.matmul(
                                av[:, off:off + 129],
                                pt8_v[:, :, s * 128:(s + 1) * 128],
                                va8_v[:, slot:slot + 2, h, 0:129],
                                start=(g == 0 and s % 2 == 0),
                                stop=(g == NGRP - 1 and s % 2 == 1),
                                perf_mode=DR,
                            )
                    if g == NGRP - 1:
                        # normalize now (DVE only); transposes are deferred
                        for t_i, av in ((0, avA), (1, avB)):
                            av_v = av[:].rearrange("p (r q) -> p r q", q=256)
                            rec2 = sp.tile([128, 2], F32, tag="rec")
                            nc.vector.reciprocal(rec2[:].unsqueeze(-1),
                                                 av_v[:, :, 128:129])
                            nc.vector.tensor_tensor(
                                oh_v[:, 2 * t_i:2 * t_i + 2, :],
                                av_v[:, :, 0:128],
                                rec2[:].unsqueeze(-1).broadcast_to([128, 2, 128]),
                                ALU.mult)
                        deferred = (qc, h, oh_v)
                flush_block(deferred)
                ln_half(1, po_cur[0], po_cur[1])

    nc.compile()
    return nc


@lru_cache(maxsize=2)
def _program(affine: bool = False):
    return _build_program(affine)


class _Executor:
    """Caches the jitted shard_map executable across kernel() calls."""

    def __init__(self, nc):
        import jax
        import concourse.mybir as mb
        from concourse import bass2jax
        from jax.sharding import Mesh, PartitionSpec
        from jax.experimental.shard_map import shard_map

        bass2jax.install_neuronx_cc_hook()
        self.jax = jax
        partition_name = (
            nc.partition_id_tensor.name if nc.partition_id_tensor else None
        )
        in_names, out_names, out_avals, zero_shapes = [], [], [], []
        for alloc in nc.m.functions[0].allocations:
            if not isinstance(alloc, mb.MemoryLocationSet):
                continue
            name = alloc.memorylocations[0].name
            if alloc.kind == "ExternalInput":
                if name != partition_name:
                    in_names.append(name)
            elif alloc.kind == "ExternalOutput":
                out_names.append(name)
                shape = tuple(alloc.tensor_shape)
                dtype = mb.dt.np(alloc.dtype)
                out_avals.append(jax.core.ShapedArray(shape, dtype))
                zero_shapes.append((shape, dtype))
        self.n_params = len(in_names)
        self.in_names = list(in_names)
        self.out_names = out_names
        self.out_avals = out_avals
        self.zero_shapes = zero_shapes
        all_in = in_names + out_names + ([partition_name] if partition_name else [])
        donate = tuple(range(self.n_params, self.n_params + len(out_names)))

        def _body(*args):
            operands = list(args)
            if partition_name is not None:
                operands.append(bass2jax.partition_id_tensor())
            return tuple(bass2jax._bass_exec_p.bind(
                *operands,
                out_avals=tuple(out_avals),
                in_names=tuple(all_in),
                out_names=tuple(out_names),
                lowering_input_output_aliases=(),
                sim_require_finite=True,
                sim_require_nnan=True,
                nc=nc,
            ))

        devices = jax.devices()[:NCORES]
        mesh = Mesh(np.asarray(devices), ("core",))
        n_in = self.n_params + len(out_names)
        self.sharded = jax.jit(
            shard_map(_body, mesh=mesh,
                      in_specs=(PartitionSpec("core"),) * n_in,
                      out_specs=(PartitionSpec("core"),) * len(out_names),
                      check_rep=False),
            donate_argnums=donate, keep_unused=True,
        )

    def concat_inputs(self, in_maps):
        return [
            np.concatenate([np.asarray(m[name]) for m in in_maps], axis=0)
            for name in self.in_names
        ]

    def zeros(self):
        return [np.zeros((NCORES * s[0], *s[1:]), d) for s, d in self.zero_shapes]

    def run(self, concat_in):
        out_arrs = self.sharded(*concat_in, *self.zeros())
        return out_arrs

    def split(self, out_arrs):
        return [
            {name: np.asarray(out_arrs[i]).reshape(NCORES, *self.out_avals[i].shape)[c]
             for i, name in enumerate(self.out_names)}
            for c in range(NCORES)
        ]


@lru_cache(maxsize=2)
def _executor(affine: bool = False):
    return _Executor(_program(affine))


def _prep_core_inputs(core, x, attn_mask, node_mask, Wq, Wk, Wv, Wo, bo,
                      gamma, beta):
    b, half = core // 2, core % 2
    rsl = slice(half * NQ, (half + 1) * NQ)
    xb = np.ascontiguousarray(x[b])
    m = {}
    m["xT"] = np.ascontiguousarray(xb.T)
    m["xqT"] = np.ascontiguousarray(xb[rsl].T)
    m["xres"] = np.ascontiguousarray(
        (xb[rsl] + bo).reshape(8, 128, 128).transpose(1, 0, 2)
    )
    mT = np.where(attn_mask[b].T[:, rsl], 0.0, -240.0).astype(np.float32)
    m["maskb"] = np.ascontiguousarray(
        mT.reshape(NMC, 128, NQ).transpose(1, 0, 2)
    ).astype(ml_dtypes.float8_e4m3)
    m["wq"], m["wk"], m["wv"] = Wq, Wk, Wv
    m["wo"] = np.ascontiguousarray(
        Wo.reshape(8, 128, 128).transpose(1, 0, 2)).astype(ml_dtypes.bfloat16)
    m["gb"] = np.ascontiguousarray(np.stack([gamma, beta]))
    m["nm"] = np.ascontiguousarray(
        node_mask[b, rsl].astype(np.float32).reshape(8, 128).T
    )
    return m


def kernel(x, attn_mask, node_mask, Wq, Wk, Wv, Wo, bo, gamma, beta):
    x = np.asarray(x, np.float32)
    attn_mask = np.asarray(attn_mask, bool)
    node_mask = np.asarray(node_mask, bool)
    Wq = np.ascontiguousarray(np.asarray(Wq, np.float32))
    Wk = np.ascontiguousarray(np.asarray(Wk, np.float32))
    Wv = np.ascontiguousarray(np.asarray(Wv, np.float32))
    Wo = np.asarray(Wo, np.float32)
    bo = np.asarray(bo, np.float32)
    gamma = np.asarray(gamma, np.float32)
    beta = np.asarray(beta, np.float32)

    affine = not (np.all(gamma == 1.0) and np.all(beta == 0.0))
    ex = _executor(affine)
    in_maps = [
        _prep_core_inputs(c, x, attn_mask, node_mask, Wq, Wk, Wv, Wo, bo,
                          gamma, beta)
        for c in range(NCORES)
    ]
    results = ex.split(ex.run(ex.concat_inputs(in_maps)))
    out = np.empty((B, N, D), np.float32)
    for core in range(NCORES):
        b, half = core // 2, core % 2
        o = results[core]["out"]  # [128, 8, 128]
        out[b, half * NQ:(half + 1) * NQ] = (
            o.transpose(1, 0, 2).reshape(NQ, 128)
        )
    return out
